# revision 1
# baseline (speedup 1.0000x reference)
"""Trainium2 Bass kernel for nn_Cross_Attention (B=8, N=2048, D=768).

Math (per batch b):
    sf  = softmax(t, axis=-1)            # t in {x2, x3}
    q   = softmax(t, axis=0)             # over tokens
    attn_i = (x @ sf_i^T) @ q_i = x @ KQ_i      with KQ_i = sf_i^T @ q_i  [D, D]
    out = f*(attn_1 @ W1^T + b1) + f*(attn_2 @ W2^T + b2) + x
        = x @ Msum + fb + x
    Msum = f*(KQ_1 @ W1^T + KQ_2 @ W2^T),  fb = f*(b1 + b2),  f = sigmoid(w)

so the [N, N] context matrix is never materialized.  With E = exp(t)
(no max-subtraction needed for randn inputs), R = rowsum(E),
S = colsum(E), G = E / sqrt(R):
    KQ[d, d'] = (sum_m G[m, d] G[m, d']) / S[d']
one scaled copy of E suffices and the 1/S rescale rides the PSUM->SBUF
copy of the KQ^T tiles.

Distribution: pure data-parallel — batch b -> core b (8 cores, no
collectives).  All matmuls run as float32r (FP22 multiply, FP32
accumulate) at full PE rate.

Schedule notes (from NTFF profiles):
  - colsum runs as ones-matmuls threaded through the exp phase to keep
    the PE warm (HAM) and fill its idle window;
  - sqrt(R) is batched into one [P, NT] activation to avoid per-tile
    ACT table reloads (Exp<->Sqrt ping-pong cost 41us);
  - x^T is produced on the fly in the y phase (PE transposes one tile
    ahead of the matmuls), so no 48KB x^T buffer exists;
  - the residual +x rides a gpsimd accumulate-DMA instead of a second
    DVE pass.
"""

import numpy as np

import concourse.bass as bass
import concourse.tile as tile
from concourse import bacc
from concourse import mybir
from concourse.bass_utils import run_bass_kernel_spmd

F32 = mybir.dt.float32
F32R = mybir.dt.float32r
BF16 = mybir.dt.bfloat16

B = 8
P = 128
D = 768
DT = D // P  # 6 feature tiles
# moving-dim chunks: each must stay inside one PSUM bank (512 f32) and be
# >=256 for the float32r full-rate path
CHUNKS = ((0, 512), (512, 256))
# upper-triangle chunk plan per d'-tile: cover columns >= dp*128, each
# chunk >=256 wide (f32r full rate) and inside one PSUM bank
SYM_CHUNKS = {
    0: ((0, 512), (512, 256)),
    1: ((128, 384), (512, 256)),
    2: ((256, 256), (512, 256)),
    3: ((256, 256), (512, 256)),
    4: ((512, 256),),
    5: ((512, 256),),
}
SYM_FILLS = [
    (1, 0), (2, 0), (2, 1), (3, 0), (3, 1),
    (4, 0), (4, 1), (4, 2), (4, 3),
    (5, 0), (5, 1), (5, 2), (5, 3),
]


def build_nc(NT=16):
    """Build the single-core Bass program.  NT = number of 128-token tiles."""
    N = NT * P
    nc = bacc.Bacc()

    x_d = nc.dram_tensor("x", [N, D], F32, kind="ExternalInput")
    x2_d = nc.dram_tensor("x2", [N, D], F32, kind="ExternalInput")
    x3_d = nc.dram_tensor("x3", [N, D], F32, kind="ExternalInput")
    wt1_d = nc.dram_tensor("wt1", [D, D], F32, kind="ExternalInput")  # f*W1^T
    wt2_d = nc.dram_tensor("wt2", [D, D], F32, kind="ExternalInput")  # f*W2^T
    fb_d = nc.dram_tensor("fb", [1, D], F32, kind="ExternalInput")  # f*(b1+b2)
    id_d = nc.dram_tensor("ident", [P, P], F32, kind="ExternalInput")  # np.eye
    out_d = nc.dram_tensor("out", [N, D], F32, kind="ExternalOutput")

    x_t = x_d.rearrange("(t p) d -> t p d", p=P)
    att_t = [
        x2_d.rearrange("(t p) d -> t p d", p=P),
        x3_d.rearrange("(t p) d -> t p d", p=P),
    ]
    out_t = out_d.rearrange("(t p) d -> t p d", p=P)

    with tile.TileContext(nc) as tc:
        with (
            tc.tile_pool(name="consts", bufs=1) as consts,
            tc.tile_pool(name="big", bufs=2) as big,
            tc.tile_pool(name="stream", bufs=3) as stream,
            tc.tile_pool(name="stats", bufs=2) as stats,
            tc.tile_pool(name="xtip", bufs=3) as xtip,
            tc.tile_pool(name="outp", bufs=3) as outp,
            tc.tile_pool(name="acc", bufs=3, space="PSUM") as acc,
            tc.tile_pool(name="tp", bufs=2, space="PSUM") as tpp,
        ):
            ones = consts.tile([P, P], BF16)
            nc.vector.memset(ones, 1.0)
            ident = consts.tile([P, P], F32)
            nc.sync.dma_start(out=ident, in_=id_d[:, :])
            identr = consts.tile([P, P], F32R)
            nc.vector.tensor_copy(identr, ident)
            fbb = consts.tile([P, D], F32)
            nc.sync.dma_start(out=fbb, in_=fb_d[0:1, :].to_broadcast([P, D]))
            # scaled KQ^T per attention: kqt[t][:, dp, d] (d' on partitions)
            kqt = [
                consts.tile([P, DT, D], F32R, tag=f"kqt{t}", name=f"kqt{t}")
                for t in range(2)
            ]
            msum = consts.tile([P, DT, D], F32R)
            # 1/S column vectors, per attention and d'-tile
            rscol = consts.tile([P, 2, DT], F32)

            # x^T helper used by the y phase; the first three tiles run
            # at the very start to give the cold PE work during exp(x2)
            def load_and_transpose(i, pre=False):
                xi = stream.tile([P, D], F32, tag="in", name=f"xi{i}")
                if pre:
                    nc.gpsimd.dma_start(out=xi, in_=x_t[i])
                else:
                    nc.sync.dma_start(out=xi, in_=x_t[i])
                xti = xtip.tile([P, DT, P], F32R, tag="xti", name=f"xti{i}")
                for c in range(DT):
                    tp = tpp.tile([P, P], F32, tag="tp", name=f"xtp{i}_{c}")
                    nc.tensor.transpose(tp, xi[:, c * P : (c + 1) * P], ident)
                    if pre:
                        nc.vector.tensor_copy(xti[:, c, :], tp)
                    else:
                        nc.any.tensor_copy(xti[:, c, :], tp)
                xtis.append(xti)
                return xi

            xtis = []
            xi_live = {}
            PRE = min(3, NT)
            for i in range(PRE):
                xi_live[i] = load_and_transpose(i, pre=True)

            # ---- per-attention phases: exp / colsum / G / KQ^T ----
            # G lives in bf16: the softmax normalizations cancel the
            # truncation bias, and bf16 streams at full PE rate (f32r
            # moving operands run at ~2 cycles/column on HW).
            for t in range(2):
                g = big.tile([P, NT, D], BF16, tag="big", name=f"g{t}")
                rvec = stats.tile([P, NT], F32, tag="rvec")
                s_ps = acc.tile([P, D], F32, tag="acc", name=f"s_ps{t}")
                for i in range(NT):
                    xi = stream.tile([P, D], F32, tag="in")
                    nc.sync.dma_start(out=xi, in_=att_t[t][i])
                    nc.scalar.activation(
                        out=g[:, i, :], in_=xi,
                        func=mybir.ActivationFunctionType.Exp,
                        accum_out=rvec[:, i : i + 1],
                    )
                    # token-direction column sums accumulate on the PE,
                    # keeping it busy/warm through the exp phase
                    for off, sz in CHUNKS:
                        nc.tensor.matmul(
                            s_ps[:, off : off + sz],
                            ones,
                            g[:, i, off : off + sz],
                            start=(i == 0), stop=(i == NT - 1),
                        )
                # batched 1/sqrt(R): one table load instead of NT
                nc.scalar.sqrt(rvec, rvec)
                nc.vector.reciprocal(rvec, rvec)
                for i in range(NT):
                    nc.vector.tensor_scalar_mul(
                        g[:, i, :], g[:, i, :], rvec[:, i : i + 1]
                    )

                rsb = stream.tile([P, D], F32, tag="rsb", bufs=2)
                nc.vector.reciprocal(rsb, s_ps)
                # transpose 1/S into per-partition column scalars
                for c in range(DT):
                    tp = tpp.tile([P, P], F32, tag="tp")
                    nc.tensor.transpose(tp, rsb[:, c * P : (c + 1) * P], ident)
                    nc.vector.tensor_copy(rscol[:, t, c : c + 1], tp[:, 0:1])

                # S values (broadcast) in SBUF for the symmetric fills
                ssb = stream.tile([P, D], F32, tag="ssb", bufs=2)
                nc.vector.tensor_copy(ssb, s_ps)

                # KQ^T_raw[d', d] = sum_m G[m, d'] G[m, d] is symmetric:
                # compute the upper block-triangle only, scale rows by
                # 1/S[d'] on the PSUM->SBUF copy
                for dp in range(DT):
                    kq_ps = acc.tile([P, D], F32, tag="acc")
                    for i in range(NT):
                        lhsT = g[:, i, dp * P : (dp + 1) * P]
                        for off, sz in SYM_CHUNKS[dp]:
                            nc.tensor.matmul(
                                kq_ps[:, off : off + sz],
                                lhsT,
                                g[:, i, off : off + sz],
                                start=(i == 0), stop=(i == NT - 1),
                            )
                    lo0 = SYM_CHUNKS[dp][0][0]
                    nc.vector.tensor_scalar_mul(
                        kqt[t][:, dp, lo0:],
                        kq_ps[:, lo0:],
                        rscol[:, t, dp : dp + 1],
                    )
                # lower blocks = transposed upper blocks rescaled:
                # kqt[hi][p, lo*P+q] = tp[p, q] * S[lo*P+q] / S[hi*P+p]
                for hi, lo in SYM_FILLS:
                    tp = tpp.tile([P, P], F32, tag="tp", name=f"sf{t}_{hi}_{lo}")
                    nc.tensor.transpose(
                        tp.bitcast(F32R),
                        kqt[t][:, lo, hi * P : (hi + 1) * P],
                        identr,
                    )
                    nc.vector.scalar_tensor_tensor(
                        out=kqt[t][:, hi, lo * P : (lo + 1) * P],
                        in0=tp,
                        scalar=rscol[:, t, hi : hi + 1],
                        in1=ssb[:, lo * P : (lo + 1) * P],
                        op0=mybir.AluOpType.mult,
                        op1=mybir.AluOpType.mult,
                    )

            # ---- weights (reuses G1's slot; DMA overlaps KQt2) ----
            wts = big.tile([P, 2, DT, D], F32R, tag="big")
            for t, wd in enumerate((wt1_d, wt2_d)):
                # gpsimd DMA casts f32 -> f32r (rounds) during the transfer
                nc.gpsimd.dma_start(
                    out=wts[:, t], in_=wd.rearrange("(c p) j -> p c j", p=P)
                )

            # ---- Msum[d, j] = sum_t sum_d' KQt[t][d', d] * wts[t][d', j] ----
            for d in range(DT):
                m_ps = acc.tile([P, D], F32, tag="acc")
                for t in range(2):
                    for dp in range(DT):
                        lhsT = kqt[t][:, dp, d * P : (d + 1) * P]
                        for off, sz in CHUNKS:
                            nc.tensor.matmul(
                                m_ps[:, off : off + sz],
                                lhsT,
                                wts[:, t, dp, off : off + sz],
                                start=(t == 0 and dp == 0),
                                stop=(t == 1 and dp == DT - 1),
                            )
                nc.any.tensor_copy(msum[:, d, :], m_ps)

            # ---- y = x @ Msum; out = y + fb + x ----
            # x^T tiles are produced on the fly, three tiles ahead of the
            # matmuls that consume them (tiles 0-2 were made at the start).
            for i in range(NT):
                if i + PRE < NT:
                    xi_live[i + PRE] = load_and_transpose(i + PRE)
                xti = xtis[i]
                y_ps = acc.tile([P, D], F32, tag="acc")
                for k in range(DT):
                    for off, sz in CHUNKS:
                        nc.tensor.matmul(
                            y_ps[:, off : off + sz],
                            xti[:, k, :],
                            msum[:, k, off : off + sz],
                            start=(k == 0), stop=(k == DT - 1),
                        )
                oi = outp.tile([P, D], F32, tag="out")
                nc.vector.tensor_add(oi, y_ps, fbb)
                if i >= NT - 2:
                    # tail tiles: finish on the DVE (x tile still resident)
                    # to avoid the accumulate-DMA round-trip at the end
                    nc.vector.tensor_add(oi, oi, xi_live[i])
                else:
                    # residual +x via accumulate-DMA (reads x from HBM
                    # again, saving a DVE pass)
                    nc.gpsimd.dma_start(
                        out=oi, in_=x_t[i], accum_op=mybir.AluOpType.add
                    )
                nc.sync.dma_start(out=out_t[i], in_=oi)

    nc.compile()
    return nc


def prep_inputs(inputs):
    x = np.ascontiguousarray(np.asarray(inputs["x"], dtype=np.float32))
    x2 = np.ascontiguousarray(np.asarray(inputs["x2"], dtype=np.float32))
    x3 = np.ascontiguousarray(np.asarray(inputs["x3"], dtype=np.float32))
    W1 = np.asarray(inputs["W1"], dtype=np.float32)
    b1 = np.asarray(inputs["b1"], dtype=np.float32)
    W2 = np.asarray(inputs["W2"], dtype=np.float32)
    b2 = np.asarray(inputs["b2"], dtype=np.float32)
    w = np.asarray(inputs["w"], dtype=np.float32)

    f = 1.0 / (1.0 + np.exp(-float(w.reshape(-1)[0])))
    wt1 = np.ascontiguousarray((f * W1).T.astype(np.float32))
    wt2 = np.ascontiguousarray((f * W2).T.astype(np.float32))
    fb = (f * (b1 + b2)).astype(np.float32).reshape(1, D)

    ident = np.eye(P, dtype=np.float32)
    return [
        {
            "x": x[b], "x2": x2[b], "x3": x3[b],
            "wt1": wt1, "wt2": wt2, "fb": fb, "ident": ident,
        }
        for b in range(B)
    ]


_NC = None


def kernel(**inputs) -> np.ndarray:
    global _NC
    if _NC is None:
        _NC = build_nc()
    in_maps = prep_inputs(inputs)
    res = run_bass_kernel_spmd(_NC, in_maps, list(range(B)))
    return np.stack([res.results[b]["out"] for b in range(B)], axis=0).astype(
        np.float32
    )

